# revision 42
# baseline (speedup 1.0000x reference)
"""Trainium2 Bass kernel for nn_AFFN (autoregressive FFN with block-triangular weights).

Math: the reference's sequential scan over L sites is only apparently sequential --
causality is baked into the (already masked) block-triangular weights. So:
    h0 = x_in_onehot @ W0f + b0 ; a1 = elu(h0)
    h1 = a1 @ W1f + b1         ; a2 = elu(h1)
    h2 = a2 @ W2f + b2         ; out = elu(h2)
    logp[b] = sum_j log_softmax(out[b,j,:])[x[b,j]]
All matmuls are dense feed-forward; the triangular structure lets us skip reading /
multiplying the zero (k > j) blocks of W1/W2.

Sharding: pure data parallel. Batch 4096 -> 512 per core; weights replicated.
Compute dtype: float32r (TF32-like PE mode, ~261ns per 128x128x512 matmul, no casts).

elu(x) = min(exp(x),1) - 1 + max(x,0)   (exp overflow-safe: inf -> min picks 1)
Final layer uses the shifted form elu+1 (log_softmax shift-invariant).
"""

import numpy as np

L = 64
H = 64
IN = 4
B = 4096
NCORES = 8
BS = B // NCORES          # 512 batch rows per core
NJB = 32                  # feature tiles of 128 = 2 sites x 64
NG = 16                   # layer-1 j-groups of 4 sites (2 jb tiles)

_CACHE = {}


def _build():
    import concourse.tile as tile
    import concourse.mybir as mybir
    from concourse import bacc

    f32 = mybir.dt.float32
    f32r = mybir.dt.float32r
    i32 = mybir.dt.int32
    Exp = mybir.ActivationFunctionType.Exp
    Ln = mybir.ActivationFunctionType.Ln
    add = mybir.AluOpType.add
    amax = mybir.AluOpType.max
    amin = mybir.AluOpType.min
    mult = mybir.AluOpType.mult
    subtract = mybir.AluOpType.subtract
    is_equal = mybir.AluOpType.is_equal

    nc = bacc.Bacc("TRN2", target_bir_lowering=False, debug=False)

    xt = nc.dram_tensor("xt", [L, BS], i32, kind="ExternalInput").ap()
    xb = nc.dram_tensor("xb", [BS, L], i32, kind="ExternalInput").ap()
    W0 = nc.dram_tensor("W0", [L, IN, L, H], f32r, kind="ExternalInput").ap()
    W1 = nc.dram_tensor("W1", [L, H, L, H], f32r, kind="ExternalInput").ap()
    W2 = nc.dram_tensor("W2", [L, H, L, IN], f32r, kind="ExternalInput").ap()
    b0t = nc.dram_tensor("b0t", [128, 32], f32, kind="ExternalInput").ap()
    b1t = nc.dram_tensor("b1t", [128, 32], f32, kind="ExternalInput").ap()
    b2t = nc.dram_tensor("b2t", [128, 2], f32, kind="ExternalInput").ap()
    out = nc.dram_tensor("out", [BS], f32, kind="ExternalOutput").ap()

    with tile.TileContext(nc) as tc:
        with (
            tc.tile_pool(name="singles", bufs=1) as singles,
            tc.tile_pool(name="a1p", bufs=NJB) as a1p,
            tc.tile_pool(name="a2p", bufs=4) as a2p,
            tc.tile_pool(name="w1p", bufs=4) as w1p,
            tc.tile_pool(name="w2p", bufs=2) as w2p,
            tc.tile_pool(name="tmp", bufs=3) as tmp,
            tc.tile_pool(name="ps2", bufs=1, space="PSUM") as ps2,
        ):
            # ---- constants / small inputs ----
            sxt = singles.tile([L, BS], i32)
            nc.sync.dma_start(sxt[:], xt)
            b0sb = singles.tile([128, 32], f32)
            nc.sync.dma_start(b0sb[:], b0t)
            b1sb = singles.tile([128, 32], f32)
            nc.sync.dma_start(b1sb[:], b1t)

            # W0 staged fully: 2 tiles (128 = (i-pair, k), 4096 = (j, s))
            w0sb = [singles.tile([128, NJB * 128], f32r, name=f"w0sb{t}")
                    for t in range(2)]
            w0q = [0]  # next 8-site W0 column chunk to fetch

            def fetch_w0_upto(q_needed):
                while w0q[0] <= min(q_needed, 7):
                    q = w0q[0]
                    eng = nc.scalar if q < 2 else nc.sync
                    for t in range(2):
                        for il in range(2):
                            eng.dma_start(
                                w0sb[t][64 * il:64 * (il + 1),
                                        512 * q:512 * (q + 1)],
                                W0[:, 2 * t + il, 8 * q:8 * (q + 1), :]
                                .rearrange("k j s -> k (j s)"),
                            )
                    w0q[0] += 1

            # one-hot of shifted x in (i, k) feature layout: 2 tiles (128, BS)
            x1h = []
            for t in range(2):
                xo = singles.tile([128, BS], f32r, name=f"x1h{t}")
                nc.vector.tensor_scalar(
                    out=xo[0:64, :], in0=sxt[:], scalar1=float(2 * t),
                    scalar2=None, op0=is_equal)
                nc.vector.tensor_scalar(
                    out=xo[64:128, :], in0=sxt[:], scalar1=float(2 * t + 1),
                    scalar2=None, op0=is_equal)
                x1h.append(xo)

            # epilogue-only constants (loaded lazily, first use at group ~6)
            from concourse.masks import make_identity
            epi_consts = {}

            def load_epi_consts():
                if epi_consts:
                    return
                ident = singles.tile([128, 128], f32, name="ident")
                make_identity(nc, ident[:])
                b2sb = singles.tile([128, 2], f32, name="b2sb")
                nc.scalar.dma_start(b2sb[:], b2t)
                xb_sb = singles.tile([128, BS // 128, L], i32, name="xb_sb")
                for c in range(BS // 128):
                    nc.scalar.dma_start(xb_sb[:, c, :], xb[128 * c:128 * (c + 1), :])
                mks = []
                for t in range(2):
                    mk = singles.tile([128, 4, 32, 4], f32, name=f"mk{t}")
                    for s in range(4):
                        nc.vector.tensor_scalar(
                            out=mk[:, :, :, s],
                            in0=xb_sb[:, :, 32 * t:32 * (t + 1)],
                            scalar1=float(s), scalar2=None, op0=is_equal)
                    mks.append(mk)
                epi_consts.update(ident=ident, b2sb=b2sb, xb_sb=xb_sb, mks=mks)

            # persistent layer-2 psum accumulators (feature-major: 128 feats x BS)
            psum2 = [ps2.tile([128, BS], f32, name=f"psum2_{t}") for t in range(2)]

            a1 = [None] * NJB

            def elu_from_psum(psum, bias_col, out_pool, name, tag, shifted=False,
                              relu_on_act=False, add_on_gp=False):
                """true elu: min(exp(h+b),1)-1+max(h+b,0); shifted drops the -1."""
                e = tmp.tile([128, BS], mybir.dt.bfloat16, name="e_t", tag="e_t")
                nc.scalar.activation(e[:], psum[:], Exp, bias=bias_col, scale=1.0)
                r = tmp.tile([128, BS], mybir.dt.bfloat16, name="r_t", tag="r_t")
                if relu_on_act:
                    nc.scalar.activation(r[:], psum[:],
                                         mybir.ActivationFunctionType.Relu,
                                         bias=bias_col, scale=1.0)
                else:
                    nc.vector.tensor_scalar(
                        out=r[:], in0=psum[:], scalar1=bias_col, scalar2=0.0,
                        op0=add, op1=amax)
                a = out_pool.tile([128, BS], f32r, name=name, tag=tag)
                if shifted:
                    nc.vector.scalar_tensor_tensor(
                        out=a[:], in0=e[:], scalar=1.0, in1=r[:],
                        op0=amin, op1=add)
                else:
                    t_ = tmp.tile([128, BS], mybir.dt.bfloat16, name="t_t", tag="t_t")
                    nc.vector.tensor_scalar(
                        out=t_[:], in0=e[:], scalar1=1.0, scalar2=-1.0,
                        op0=amin, op1=add)
                    if add_on_gp:
                        nc.gpsimd.tensor_tensor(a[:], t_[:], r[:], add)
                    else:
                        nc.vector.tensor_add(a[:], t_[:], r[:])
                return a

            with (
                tc.tile_pool(name="ps0", bufs=2, space="PSUM") as ps0,
                tc.tile_pool(name="ps1", bufs=4, space="PSUM") as ps1,
                tc.tile_pool(name="epi", bufs=1) as epi,
            ):
                emitted_l0 = [0]  # next jb to emit
                w2cs = []
                lpacc = singles.tile([128, BS // 128], f32)

                def emit_layer0_upto(jb_max):
                    while emitted_l0[0] <= min(jb_max, NJB - 1):
                        jb = emitted_l0[0]
                        p0 = ps0.tile([128, BS], f32, name="p0", tag="p0")
                        nc.tensor.matmul(
                            p0[:], w0sb[0][:, 128 * jb:128 * (jb + 1)], x1h[0][:],
                            start=True, stop=False)
                        nc.tensor.matmul(
                            p0[:], w0sb[1][:, 128 * jb:128 * (jb + 1)], x1h[1][:],
                            start=False, stop=True)
                        a1[jb] = elu_from_psum(p0, b0sb[:, jb:jb + 1], a1p,
                                               f"a1_{jb}", "a1")
                        emitted_l0[0] += 1

                def emit_epilogue_half(t):
                    """log-softmax for j in [32t, 32t+32) from psum2[t] (batched
                    over all 4 batch chunks via transposed layout)."""
                    ident, b2sb, xb_sb, mks = (
                        epi_consts[k] for k in ("ident", "b2sb", "xb_sb", "mks"))
                    h = epi.tile([128, BS], f32, name=f"hb{t}", tag=f"hb{t}")
                    nc.vector.tensor_scalar(
                        out=h[:], in0=psum2[t][:], scalar1=b2sb[:, t:t + 1],
                        scalar2=None, op0=add)
                    oT = epi.tile([128, 4, 128], f32, name="oT", tag="oT")
                    for c in range(4):
                        ptr = ps0.tile([128, BS], f32, name="p0t", tag="p0")
                        nc.tensor.transpose(
                            ptr[:, 0:128], h[:, 128 * c:128 * (c + 1)], ident[:])
                        nc.vector.tensor_copy(oT[:, c, :], ptr[:, 0:128])
                    flat = oT[:].rearrange("p c f -> p (c f)")
                    oc = epi.tile([128, 512], f32, name="oc", tag="oc")
                    nc.vector.tensor_scalar(
                        out=oc[:], in0=flat, scalar1=80.0, scalar2=None, op0=amin)
                    e = epi.tile([128, 512], f32, name="e_ep", tag="e_ep")
                    nc.scalar.activation(e[:], oc[:], Exp)
                    t1 = epi.tile([128, 512], f32, name="t1_ep", tag="t1_ep")
                    nc.vector.tensor_scalar(
                        out=t1[:], in0=e[:], scalar1=1.0, scalar2=-1.0,
                        op0=amin, op1=add)
                    v = epi.tile([128, 4, 32, 4], f32, name="v_ep", tag="v_ep")
                    nc.vector.scalar_tensor_tensor(
                        out=v[:].rearrange("p c j s -> p (c j s)"), in0=flat,
                        scalar=0.0, in1=t1[:], op0=amax, op1=add)
                    m = epi.tile([128, 4, 32], f32, name="m_ep", tag="m_ep")
                    nc.vector.tensor_reduce(
                        out=m[:], in_=v[:], axis=mybir.AxisListType.X, op=amax)
                    z = epi.tile([128, 4, 32, 4], f32, name="z_ep", tag="z_ep")
                    nc.vector.tensor_tensor(
                        z[:], v[:], m[:, :, :, None].to_broadcast((128, 4, 32, 4)),
                        subtract)
                    E = epi.tile([128, 4, 32, 4], f32, name="E_ep", tag="E_ep")
                    nc.scalar.activation(E[:].rearrange("p c j s -> p (c j s)"),
                                         z[:].rearrange("p c j s -> p (c j s)"), Exp)
                    S = epi.tile([128, 4, 32], f32, name="S_ep", tag="S_ep")
                    nc.vector.tensor_reduce(
                        out=S[:], in_=E[:], axis=mybir.AxisListType.X, op=add)
                    Lg = epi.tile([128, 4, 32], f32, name="Lg_ep", tag="Lg_ep")
                    nc.scalar.activation(
                        Lg[:].rearrange("p c j -> p (c j)"),
                        S[:].rearrange("p c j -> p (c j)"), Ln)
                    mk = mks[t]
                    vm = epi.tile([128, 4, 32, 4], f32, name="vm_ep", tag="vm_ep")
                    nc.vector.tensor_tensor(vm[:], z[:], mk[:], mult)
                    selz = epi.tile([128, 4, 32], f32, name="selz_ep", tag="selz_ep")
                    nc.vector.tensor_reduce(
                        out=selz[:], in_=vm[:], axis=mybir.AxisListType.X, op=add)
                    d = epi.tile([128, 4, 32], f32, name="d_ep", tag="d_ep")
                    nc.vector.tensor_tensor(d[:], selz[:], Lg[:], subtract)
                    if t == 0:
                        nc.vector.tensor_reduce(
                            out=lpacc[:], in_=d[:], axis=mybir.AxisListType.X, op=add)
                    else:
                        lp1 = epi.tile([128, 4], f32, name="lp1", tag="lp1")
                        nc.vector.tensor_reduce(
                            out=lp1[:], in_=d[:], axis=mybir.AxisListType.X, op=add)
                        nc.vector.tensor_add(lpacc[:], lpacc[:], lp1[:])
                        nc.sync.dma_start(
                            out.rearrange("(c p) -> p c", p=128), lpacc[:])

                NG8 = 8  # 8-site layer-1 groups (4 jb tiles each)
                for g8 in range(NG8):
                    if g8 == 3:
                        load_epi_consts()
                    jbs = [4 * g8 + m_ for m_ in range(4)]
                    nkb = jbs[-1] + 1

                    # layer 0 lookahead first (its DMAs/matmuls gate the
                    # wavefront). Keep it tight early: deep lookahead just
                    # serializes PE on the 2-slot ps0 recycle before group
                    # matmuls exist to hide the elu latency.
                    la = 4 * g8 + (3 if g8 <= 1 else 5)
                    fetch_w0_upto(la // 4)
                    emit_layer0_upto(la)

                    # W1 stream: fused 4-kb-block DMAs, 2KB lines
                    w1ts = []
                    for c0 in range(0, nkb, 4):
                        cnt = min(4, nkb - c0)
                        w1c = w1p.tile([128, 4, 512], f32r, name="w1c", tag="w1c")
                        nc.sync.dma_start(
                            w1c[:, :cnt, :],
                            W1[2 * c0:2 * (c0 + cnt), :, 8 * g8:8 * g8 + 8, :]
                            .rearrange("(c k) i j s -> k i c (j s)", k=2),
                        )
                        for u in range(cnt):
                            w1ts.append(w1c[:, u, :])

                    # W2 chunk for these jb (kb = jb); kb>=16 contributes only
                    # to j>=32 (t=1), so read just that half of the (j,s) cols
                    if g8 % 2 == 0:
                        if g8 < 4:
                            w2c = w2p.tile([128, 8, 256], f32r, name="w2c",
                                           tag="w2c")
                            nc.sync.dma_start(
                                w2c[:],
                                W2[8 * g8:8 * g8 + 16]
                                .rearrange("(c k) i j s -> k i c (j s)", k=2))
                        else:
                            w2c = w2p.tile([128, 8, 128], f32r, name="w2ch",
                                           tag="w2ch")
                            nc.sync.dma_start(
                                w2c[:],
                                W2[8 * g8:8 * g8 + 16, :, 32:, :]
                                .rearrange("(c k) i j s -> k i c (j s)", k=2))
                        w2cs.append(w2c)

                    # --- layer 1 group: psum tiles for 4 jb ---
                    # psum2[0]'s bank is free after epilogue half 0 (group ~4);
                    # lend it to the last groups as a 5th slot to soften the
                    # group-boundary psum-recycle stall.
                    p1 = {}
                    for m_, jb in enumerate(jbs):
                        if g8 >= 6 and m_ == 0:
                            p1[jb] = ps2.tile([128, BS], f32, name="p1x",
                                              tag="psum2_0")
                        else:
                            p1[jb] = ps1.tile([128, BS], f32, name="p1", tag="p1")
                    for kb in range(nkb):
                        w1t = w1ts[kb]
                        for m_, jb in enumerate(jbs):
                            if kb <= jb:
                                nc.tensor.matmul(
                                    p1[jb][:],
                                    w1t[:, 128 * m_:128 * (m_ + 1)],
                                    a1[kb][:],
                                    start=(kb == 0), stop=(kb == jb))

                    # --- elu -> a2[jb], then layer-2 matmuls for kb=jb ---
                    for m_, jb in enumerate(jbs):
                        a2t = elu_from_psum(p1[jb], b1sb[:, jb:jb + 1], a2p,
                                            "a2t", "a2")
                        kb = jb
                        w2t = w2cs[kb // 8][:, kb % 8, :]
                        for t in range(2):
                            if kb >= 16 and t == 0:
                                continue
                            lhs = (w2t[:, 128 * t:128 * (t + 1)] if kb < 16
                                   else w2t[:, 0:128])
                            nc.tensor.matmul(
                                psum2[t][:], lhs, a2t[:],
                                start=(kb == 0),
                                stop=(kb == (15 if t == 0 else 31)))

                    if g8 == 3:
                        emit_epilogue_half(0)
                emit_epilogue_half(1)

    nc.compile()
    return nc


def _host_prep(x, W0, W1, W2, b0, b1, b2):
    x = np.ascontiguousarray(np.asarray(x, dtype=np.int32))
    W0 = np.ascontiguousarray(np.asarray(W0, dtype=np.float32))
    W1 = np.ascontiguousarray(np.asarray(W1, dtype=np.float32))
    W2 = np.ascontiguousarray(np.asarray(W2, dtype=np.float32))
    b0 = np.ascontiguousarray(np.asarray(b0, dtype=np.float32))
    b1 = np.ascontiguousarray(np.asarray(b1, dtype=np.float32))
    b2 = np.ascontiguousarray(np.asarray(b2, dtype=np.float32))

    b0t_ = np.ascontiguousarray(b0.reshape(4096).reshape(32, 128).T)
    b1t_ = np.ascontiguousarray(b1.reshape(4096).reshape(32, 128).T)
    b2t_ = np.ascontiguousarray(b2.reshape(256).reshape(2, 128).T)
    in_maps = []
    for c in range(NCORES):
        xs = x[c * BS:(c + 1) * BS]                     # (BS, L)
        xt = np.full((L, BS), -1, dtype=np.int32)
        xt[1:] = xs.T[: L - 1]
        in_maps.append({
            "xt": xt, "xb": xs,
            "W0": W0, "W1": W1, "W2": W2, "b0t": b0t_, "b1t": b1t_, "b2t": b2t_,
        })
    return in_maps


def _run(in_maps, trace=False, **kw):
    from concourse.bass_utils import run_bass_kernel_spmd
    if "nc" not in _CACHE:
        _CACHE["nc"] = _build()
    return run_bass_kernel_spmd(
        _CACHE["nc"], in_maps, core_ids=list(range(NCORES)), trace=trace, **kw)


def kernel(x, W0, W1, W2, b0, b1, b2):
    in_maps = _host_prep(x, W0, W1, W2, b0, b1, b2)
    res = _run(in_maps)
    return np.concatenate([r["out"] for r in res.results]).astype(np.float32)
